# revision 3
# baseline (speedup 1.0000x reference)
"""Trainium2 Bass kernel for nn_SSDReduceBoundingBoxes — scatter-free.

HW measurements: swdge scatter/gather Q7 jobs cost ~75ns/index (~620us for the
8064-index scatter) and dominated v1-v3 (~570-680us each). v4 eliminates ALL
Q7 scatter/gather jobs:

  - Compaction (8064 boxes -> 1024 slots) via PE selection matmuls:
    p(c) = #{p: rowoffEnd[p] <= c} (one DVE compare + ones-matmul colsum),
    S[p, c] = (p == p(c)) selection matrix, selP = S_block^T @ [scan|valid|
    prob|rx1|ry1|rx2|ry2|rowoff] gives each slot its partition's rows, then
    mask = (scanP == j+1) & validP picks the element: cmp5[c, k] =
    sum(chP * mask) via accumulating stt. Empty slots come out exactly zero.
  - Sort (1024 perm) via permutation matmuls: P2row_gc = (rank[:,gc] == iota),
    64 matmuls [128x128x5] accumulate sorted rows in PSUM.
  - Row broadcasts via PE selector matmuls (sel8), as v3.
  - Greedy fixed point: DVE + 2 PE matmuls per round, R=8.
  - Everything elementwise on DVE (same-engine chains are ~free on HW;
    DVE<->Pool and DVE<->DMA hops are ~2.7us each).

Remaining DMAs: const loads, channel loads, output writes + tail zero-fill.
No DRAM scratch at all.
"""
import numpy as np
import concourse.bass as bass
import concourse.bacc as bacc
import concourse.mybir as mybir
import concourse.tile as tile
from concourse import library_config

F32 = mybir.dt.float32
I32 = mybir.dt.int32
I16 = mybir.dt.int16
OP = mybir.AluOpType
AX = mybir.AxisListType

P = 128
T = 63
NPAD = P * T     # 8064
N = 8000
C = 1024
G = 8
PROB_TH = 0.9
R_GREEDY = 7
TWO23 = 8388608.0

# bigrhs column layout
RC_SCAN = 0          # scan (inclusive prefix of valid), 63
RC_VALID = 63        # valid mask, 63
RC_CH = 126          # 5 channels x 63: prob, rx1, ry1, rx2, ry2
RC_ROWOFF = 441      # exclusive partition offset, 1
RC_TOTAL = 442


def host_constants():
    n = np.arange(NPAD)
    lvl = (n >= 1600).astype(np.int64)
    n0 = np.where(lvl == 0, n, n - 1600)
    gp = np.where(lvl == 0, 40, 80)
    xps = np.where(lvl == 0, 16.0, 8.0)
    yps = np.where(lvl == 0, 12.0, 6.0)
    ii = n0 // gp
    jj = n0 % gp
    pad = n >= N
    iiv = np.where(pad, 0.0, ii * xps).astype(np.float32)
    jjv = np.where(pad, 0.0, jj * yps).astype(np.float32)
    xpsv = np.where(pad, 0.0, xps).astype(np.float32)
    ypsv = np.where(pad, 0.0, yps).astype(np.float32)
    tomat = lambda a: a.reshape(P, T)

    ident = np.eye(P, dtype=np.float32)
    su = (np.arange(P)[:, None] < np.arange(P)[None, :]).astype(np.float32)
    wconst = np.zeros((P, 64), dtype=np.float32)
    for p in range(P):
        for g in range(8):
            wconst[p, g * 8 + (p // 16)] = float(1 << (p % 16))
    pow2row = np.tile((1 << (np.arange(C) % 16)).astype(np.float32), (P, 1))
    t128 = ((np.arange(P)[None, :] < np.arange(P)[:, None])
            * (1 << (np.arange(P) % 16))[None, :]).astype(np.float32)
    tlones = (np.arange(P)[None, :] < np.arange(P)[:, None]).astype(np.float32)
    # sliding-window consts: cols [0:1024] full-block mask, [1024:1152] diagonal-block
    # mask; slice [1024-128*g : 1024-128*g + 128*(g+1)] gives block g's full+diag mask
    # (pow2row is 16-periodic so the shifted window matches pow2row[c] exactly)
    pwext = np.concatenate([pow2row, t128], axis=1)
    tlext = np.concatenate([np.ones((P, C), np.float32), tlones], axis=1)
    sel8 = np.zeros((8, 1024), dtype=np.float32)
    for g in range(8):
        sel8[g, 128 * g:128 * (g + 1)] = 1.0
    iota1024 = np.tile(np.arange(C, dtype=np.float32), (P, 1))
    pcolv = np.arange(P, dtype=np.float32).reshape(P, 1)
    iotapg = (np.arange(P)[:, None] + 128.0 * np.arange(G)[None, :]).astype(np.float32)
    return {
        "iiv": tomat(iiv), "jjv": tomat(jjv), "xpsv": tomat(xpsv), "ypsv": tomat(ypsv),
        "ident": ident, "su": su, "wconst": wconst, "pow2row": pow2row, "t128": t128,
        "tlones": tlones, "sel8": sel8, "iota1024": iota1024, "pcolv": pcolv,
        "iotapg": iotapg, "pwext": pwext, "tlext": tlext,
    }


def _emit_channel_loads(nc, ch, srcs):
    segs = [(0, 1600, 0, 0), (1600, 6400, 1, 0)]
    for n0, length, si, soff in segs:
        src = srcs[si]
        off = soff
        n = n0
        rem = length
        while rem > 0:
            p0, t0 = divmod(n, T)
            if t0 != 0:
                run = min(T - t0, rem)
                nc.sync.dma_start(out=ch[p0:p0 + 1, t0:t0 + run],
                                  in_=src[off:off + run].rearrange('(o a) -> o a', o=1))
            else:
                nfull = rem // T
                if nfull == 0:
                    run = rem
                    nc.sync.dma_start(out=ch[p0:p0 + 1, 0:run],
                                      in_=src[off:off + run].rearrange('(o a) -> o a', o=1))
                else:
                    run = nfull * T
                    nc.sync.dma_start(out=ch[p0:p0 + nfull, :],
                                      in_=src[off:off + run].rearrange("(a b) -> a b", b=T))
            off += run
            n += run
            rem -= run


def build(nc=None, dbg=False, repeat=1, stop_after=None):
    if nc is None:
        nc = bacc.Bacc(None, target_bir_lowering=False, debug=False)

    outs0 = nc.dram_tensor("outs0", [5, 40, 40], F32, kind="ExternalInput")
    outs1 = nc.dram_tensor("outs1", [5, 80, 80], F32, kind="ExternalInput")
    iiv_d = nc.dram_tensor("iiv", [P, T], F32, kind="ExternalInput")
    jjv_d = nc.dram_tensor("jjv", [P, T], F32, kind="ExternalInput")
    xpsv_d = nc.dram_tensor("xpsv", [P, T], F32, kind="ExternalInput")
    ypsv_d = nc.dram_tensor("ypsv", [P, T], F32, kind="ExternalInput")
    ident_d = nc.dram_tensor("ident", [P, P], F32, kind="ExternalInput")
    su_d = nc.dram_tensor("su", [P, P], F32, kind="ExternalInput")
    wconst_d = nc.dram_tensor("wconst", [P, 64], F32, kind="ExternalInput")
    pow2row_d = nc.dram_tensor("pow2row", [P, C], F32, kind="ExternalInput")
    t128_d = nc.dram_tensor("t128", [P, P], F32, kind="ExternalInput")
    tlones_d = nc.dram_tensor("tlones", [P, P], F32, kind="ExternalInput")
    sel8_d = nc.dram_tensor("sel8", [8, 1024], F32, kind="ExternalInput")
    iota1024_d = nc.dram_tensor("iota1024", [P, C], F32, kind="ExternalInput")
    pcolv_d = nc.dram_tensor("pcolv", [P, 1], F32, kind="ExternalInput")
    iotapg_d = nc.dram_tensor("iotapg", [P, G], F32, kind="ExternalInput")
    pwext_d = nc.dram_tensor("pwext", [P, C + P], F32, kind="ExternalInput")
    tlext_d = nc.dram_tensor("tlext", [P, C + P], F32, kind="ExternalInput")
    out_d = nc.dram_tensor("out", [N, 5], F32, kind="ExternalOutput")

    with tile.TileContext(nc) as tc:
        with tc.tile_pool(name="cst", bufs=1) as cst:
            nc.gpsimd.load_library(library_config.mlp)
            iiv = cst.tile([P, T], F32); nc.sync.dma_start(out=iiv[:], in_=iiv_d[:])
            jjv = cst.tile([P, T], F32); nc.sync.dma_start(out=jjv[:], in_=jjv_d[:])
            xpsv = cst.tile([P, T], F32); nc.sync.dma_start(out=xpsv[:], in_=xpsv_d[:])
            ypsv = cst.tile([P, T], F32); nc.sync.dma_start(out=ypsv[:], in_=ypsv_d[:])
            ident = cst.tile([P, P], F32); nc.sync.dma_start(out=ident[:], in_=ident_d[:])
            su = cst.tile([P, P], F32); nc.sync.dma_start(out=su[:], in_=su_d[:])
            wconst = cst.tile([P, 64], F32); nc.sync.dma_start(out=wconst[:], in_=wconst_d[:])
            pow2row = cst.tile([P, C], F32); nc.sync.dma_start(out=pow2row[:], in_=pow2row_d[:])
            t128 = cst.tile([P, P], F32); nc.sync.dma_start(out=t128[:], in_=t128_d[:])
            tlones = cst.tile([P, P], F32); nc.sync.dma_start(out=tlones[:], in_=tlones_d[:])
            sel8 = cst.tile([8, 1024], F32); nc.sync.dma_start(out=sel8[:], in_=sel8_d[:])
            iota1024 = cst.tile([P, C], F32); nc.sync.dma_start(out=iota1024[:], in_=iota1024_d[:])
            pcolv = cst.tile([P, 1], F32); nc.sync.dma_start(out=pcolv[:], in_=pcolv_d[:])
            iotapg = cst.tile([P, G], F32); nc.sync.dma_start(out=iotapg[:], in_=iotapg_d[:])
            pwext = cst.tile([P, C + P], F32); nc.sync.dma_start(out=pwext[:], in_=pwext_d[:])
            tlext = cst.tile([P, C + P], F32); nc.sync.dma_start(out=tlext[:], in_=tlext_d[:])
            ones63 = cst.tile([P, T], F32); nc.vector.memset(ones63[:], 1.0)
            onescol = cst.tile([P, 1], F32); nc.vector.memset(onescol[:], 1.0)
            onesrow = cst.tile([1, P], F32); nc.vector.memset(onesrow[:], 1.0)
            zsb = cst.tile([P, 272], F32); nc.vector.memset(zsb[:], 0.0)

            for _rep in range(repeat):
                _body(nc, tc, locals(), stop_after)
    nc.compile()
    return nc


def _body(nc, tc, env, stop_after=None):
    outs0 = env["outs0"]; outs1 = env["outs1"]
    iiv = env["iiv"]; jjv = env["jjv"]; xpsv = env["xpsv"]; ypsv = env["ypsv"]
    ident = env["ident"]; su = env["su"]; wconst = env["wconst"]
    pow2row = env["pow2row"]; t128 = env["t128"]; tlones = env["tlones"]
    sel8 = env["sel8"]; iota1024 = env["iota1024"]; pcolv = env["pcolv"]
    iotapg = env["iotapg"]; pwext = env["pwext"]; tlext = env["tlext"]
    ones63 = env["ones63"]; onescol = env["onescol"]; onesrow = env["onesrow"]
    zsb = env["zsb"]
    out_d = env["out_d"]

    with (
        tc.tile_pool(name="sb", bufs=1) as sb,
        tc.tile_pool(name="big1", bufs=1) as big1,
        tc.tile_pool(name="big2", bufs=2) as big2,
        tc.tile_pool(name="psA", bufs=1, space="PSUM") as psA,
        tc.tile_pool(name="psB", bufs=2, space="PSUM") as psB,
        tc.tile_pool(name="psC", bufs=1, space="PSUM") as psC,
        tc.tile_pool(name="psD", bufs=1, space="PSUM") as psD,
    ):
        # ---- S0: async zero-fill of the output tail ----
        outflat = out_d[:].rearrange("a b -> (a b)")
        nc.sync.dma_start(
            out=outflat[5120:39936].rearrange("(p x) -> p x", p=P), in_=zsb[:, 0:272])
        nc.sync.dma_start(
            out=outflat[39936:40000].rearrange('(o a) -> o a', o=1), in_=zsb[0:1, 0:64])

        # ---- A: channel loads ----
        o0f = outs0[:].rearrange("c a b -> c (a b)")
        o1f = outs1[:].rearrange("c a b -> c (a b)")
        chs = []
        for cch in range(5):
            ch = sb.tile([P, T], F32, name=f"ch{cch}")
            nc.vector.memset(ch[:], 0.0)
            _emit_channel_loads(nc, ch, [o0f[cch], o1f[cch]])
            chs.append(ch)
        prob, xr, yr, wr, hr = chs

        # ---- B: prep into bigrhs (all DVE) ----
        bigrhs = big1.tile([P, RC_TOTAL], F32, name="bigrhs")
        valid = bigrhs[:, RC_VALID:RC_VALID + T]
        nc.vector.tensor_scalar(out=valid, in0=prob[:], scalar1=PROB_TH,
                                scalar2=None, op0=OP.is_gt)
        nc.vector.tensor_copy(out=bigrhs[:, RC_CH:RC_CH + T], in_=prob[:])
        cx = sb.tile([P, T], F32)
        nc.vector.tensor_tensor(out=cx[:], in0=xr[:], in1=xpsv[:], op=OP.mult)
        nc.vector.tensor_tensor(out=cx[:], in0=cx[:], in1=iiv[:], op=OP.add)
        cy = sb.tile([P, T], F32)
        nc.vector.tensor_tensor(out=cy[:], in0=yr[:], in1=ypsv[:], op=OP.mult)
        nc.vector.tensor_tensor(out=cy[:], in0=cy[:], in1=jjv[:], op=OP.add)
        x2 = sb.tile([P, T], F32)
        nc.vector.scalar_tensor_tensor(out=x2[:], in0=wr[:], scalar=640.0, op0=OP.mult,
                                       in1=cx[:], op1=OP.add)
        y2 = sb.tile([P, T], F32)
        nc.vector.scalar_tensor_tensor(out=y2[:], in0=hr[:], scalar=480.0, op0=OP.mult,
                                       in1=cy[:], op1=OP.add)
        for k, v in ((1, cx), (2, cy), (3, x2), (4, y2)):
            nc.vector.tensor_scalar(out=bigrhs[:, RC_CH + T * k:RC_CH + T * (k + 1)],
                                    in0=v[:], scalar1=TWO23, scalar2=TWO23,
                                    op0=OP.add, op1=OP.subtract)

        # ---- C: prefix scan + partition offsets ----
        scan = bigrhs[:, RC_SCAN:RC_SCAN + T]
        nc.vector.tensor_tensor_scan(out=scan, data0=valid, data1=ones63[:],
                                     initial=0.0, op0=OP.add, op1=OP.mult)
        rowoff = psA.tile([P, 1], F32, space="PSUM", tag="rowoff")
        nc.tensor.matmul(out=rowoff[:], lhsT=su[:], rhs=bigrhs[:, T - 1:T],
                         start=True, stop=True)
        nc.vector.tensor_copy(out=bigrhs[:, RC_ROWOFF:RC_ROWOFF + 1], in_=rowoff[:])
        rowoffEnd = sb.tile([P, 1], F32)
        nc.vector.tensor_tensor(out=rowoffEnd[:], in0=rowoff[:],
                                in1=bigrhs[:, T - 1:T], op=OP.add)

        # ---- D: p(c) and selection matrix S ----
        Mge = big2.tile([P, C], F32, name="Mge")
        nc.vector.tensor_scalar(out=Mge[:], in0=iota1024[:], scalar1=rowoffEnd[:, 0:1],
                                scalar2=None, op0=OP.is_ge)
        pcps = psD.tile([1, C], F32, space="PSUM", tag="bigbc")
        for h in range(2):
            nc.tensor.matmul(out=pcps[:, 512 * h:512 * (h + 1)], lhsT=onescol[:],
                             rhs=Mge[:, 512 * h:512 * (h + 1)], start=True, stop=True)
        pcsb = sb.tile([1, C], F32)
        nc.vector.tensor_copy(out=pcsb[:], in_=pcps[:])
        pcRep = psD.tile([P, C], F32, space="PSUM", tag="bigbc")
        for h in range(2):
            nc.tensor.matmul(out=pcRep[:, 512 * h:512 * (h + 1)], lhsT=onesrow[:],
                             rhs=pcsb[:, 512 * h:512 * (h + 1)], start=True, stop=True)
        S = big1.tile([P, C], F32, name="Smat")
        nc.vector.tensor_scalar(out=S[:], in0=pcRep[:], scalar1=pcolv[:, 0:1],
                                scalar2=None, op0=OP.is_equal)

        # ---- E: select partition rows + pick elements -> cmp5 ----
        cmp5 = sb.tile([P, G, 5], F32)
        jcol = sb.tile([P, G], F32)
        for g in range(G):
            selps = psB.tile([P, RC_TOTAL], F32, space="PSUM", name=f"sel{g}", tag="sel")
            nc.tensor.matmul(out=selps[:], lhsT=S[:, P * g:P * (g + 1)],
                             rhs=bigrhs[:], start=True, stop=True)
            nc.vector.tensor_tensor(out=jcol[:, g:g + 1], in0=iotapg[:, g:g + 1],
                                    in1=selps[:, RC_ROWOFF:RC_ROWOFF + 1], op=OP.subtract)
            jp1 = sb.tile([P, 1], F32, name="jp1")
            nc.vector.tensor_scalar(out=jp1[:], in0=jcol[:, g:g + 1], scalar1=1.0,
                                    scalar2=None, op0=OP.add)
            mask = sb.tile([P, T], F32, name="selmask")
            nc.vector.tensor_scalar(out=mask[:], in0=selps[:, RC_SCAN:RC_SCAN + T],
                                    scalar1=jp1[:, 0:1], scalar2=None, op0=OP.is_equal)
            nc.vector.tensor_tensor(out=mask[:], in0=mask[:],
                                    in1=selps[:, RC_VALID:RC_VALID + T], op=OP.mult)
            for k in range(5):
                scr = sb.tile([P, T], F32, name="selscr")
                nc.vector.scalar_tensor_tensor(
                    out=scr[:], in0=selps[:, RC_CH + T * k:RC_CH + T * (k + 1)],
                    scalar=0.0, op0=OP.bypass, in1=mask[:], op1=OP.mult,
                    accum_out=cmp5[:, g, k:k + 1])
        cmp40 = cmp5[:].rearrange("p g k -> p (g k)")
        if stop_after == "front":
            nc.sync.dma_start(out=out_d[0:P, :], in_=cmp5[:, 0, 0:5])
            return

        # ---- F: rank by score (PE transpose + PE broadcast + DVE counts) ----
        sT = psA.tile([G, P], F32, space="PSUM", tag="trep")
        nc.tensor.transpose(out=sT[:], in_=cmp5[:, :, 0], identity=ident[:])
        sTs = sb.tile([G, P], F32)
        nc.vector.tensor_copy(out=sTs[:], in_=sT[:])
        sRep_ps = psD.tile([P, C], F32, space="PSUM", tag="bigbc")
        for g in range(G):
            nc.tensor.matmul(out=sRep_ps[:, P * g:P * (g + 1)],
                             lhsT=sel8[:, P * g:P * (g + 1)],
                             rhs=sTs[:], start=True, stop=True)
        sRep = big1.tile([P, C], F32, name="sRep")
        nc.vector.tensor_copy(out=sRep[:], in_=sRep_ps[:])
        rank_f = sb.tile([P, G], F32)
        tie = sb.tile([P, G], F32)
        for h in range(G):
            s_h = cmp40[:, 5 * h:5 * h + 1]
            Kh = P * (h + 1)
            scr = big2.tile([P, C], F32, name="rnk")
            nc.vector.scalar_tensor_tensor(
                out=scr[:], in0=sRep[:], scalar=s_h, op0=OP.is_gt,
                in1=sRep[:], op1=OP.bypass, accum_out=rank_f[:, h:h + 1])
            scr2 = big2.tile([P, C + P], F32, name="tie")
            nc.vector.scalar_tensor_tensor(
                out=scr2[:, 0:Kh], in0=sRep[:, 0:Kh], scalar=s_h,
                op0=OP.is_equal, in1=tlext[:, C - P * h:C - P * h + Kh], op1=OP.mult,
                accum_out=tie[:, h:h + 1])
        rank = sb.tile([P, G], F32)
        nc.vector.tensor_tensor(out=rank[:], in0=rank_f[:], in1=tie[:], op=OP.add)

        # ---- G: sort via permutation matmuls ----
        P2 = big1.tile([P, G, C], F32, name="P2")
        for gc in range(G):
            nc.vector.tensor_tensor(
                out=P2[:, gc, :], in0=rank[:, gc:gc + 1].to_broadcast([P, C]),
                in1=iota1024[:], op=OP.is_equal)
        srtA = psC.tile([P, 4, 5], F32, space="PSUM", tag="pk")
        srtB = psC.tile([P, 4, 5], F32, space="PSUM", tag="bcp")
        for gr in range(G):
            dst = srtA if gr % 2 == 0 else srtB
            for gc in range(G):
                nc.tensor.matmul(out=dst[:, gr // 2, :],
                                 lhsT=P2[:, gc, P * gr:P * (gr + 1)],
                                 rhs=cmp5[:, gc, :], start=(gc == 0), stop=(gc == G - 1))
        srt5 = sb.tile([P, G, 5], F32)
        nc.vector.tensor_copy(out=srt5[:].rearrange("p (h g) k -> p h g k", h=4)[:, :, 0, :],
                              in_=srtA[:])
        nc.vector.tensor_copy(out=srt5[:].rearrange("p (h g) k -> p h g k", h=4)[:, :, 1, :],
                              in_=srtB[:])
        srt40 = srt5[:].rearrange("p g k -> p (g k)")
        if stop_after == "sorted":
            nc.sync.dma_start(out=out_d[0:P, :], in_=srt5[:, 0, 0:5])
            return

        # ---- H: sorted quantities + coord broadcast via PE ----
        ss = srt5[:, :, 0]
        sx1 = srt5[:, :, 1]; sy1 = srt5[:, :, 2]; sx2 = srt5[:, :, 3]; sy2 = srt5[:, :, 4]
        svalid = sb.tile([P, G], F32)
        nc.vector.tensor_scalar(out=svalid[:], in0=ss, scalar1=PROB_TH,
                                scalar2=None, op0=OP.is_gt)
        qg = sb.tile([P, 5, G], F32)
        nc.vector.tensor_copy(out=qg[:, 0, :], in_=sx2)                  # x2
        nc.vector.tensor_scalar(out=qg[:, 1, :], in0=sx1, scalar1=-1.0,
                                scalar2=None, op0=OP.mult)               # -x1
        nc.vector.tensor_copy(out=qg[:, 2, :], in_=sy2)                  # y2
        nc.vector.tensor_scalar(out=qg[:, 3, :], in0=sy1, scalar1=-1.0,
                                scalar2=None, op0=OP.mult)               # -y1
        arw = sb.tile([P, G], F32)
        nc.vector.tensor_tensor(out=arw[:], in0=sx2, in1=sx1, op=OP.subtract)
        arh = sb.tile([P, G], F32)
        nc.vector.tensor_tensor(out=arh[:], in0=sy2, in1=sy1, op=OP.subtract)
        ar = sb.tile([P, G], F32)
        nc.vector.tensor_tensor(out=ar[:], in0=arw[:], in1=arh[:], op=OP.mult)
        nc.vector.tensor_scalar(out=qg[:, 4, :], in0=ar[:], scalar1=-1.0,
                                scalar2=None, op0=OP.mult)               # -area
        rep = big1.tile([P, 5, C], F32, name="rep")
        for q in range(5):
            qT = psA.tile([G, P], F32, space="PSUM", tag="trep")
            nc.tensor.transpose(out=qT[:], in_=qg[:, q, :], identity=ident[:])
            qTs = sb.tile([G, P], F32, name=f"qTs{q}")
            nc.vector.tensor_copy(out=qTs[:], in_=qT[:])
            rep_ps = psD.tile([P, C], F32, space="PSUM", tag="bigbc")
            for g in range(G):
                nc.tensor.matmul(out=rep_ps[:, P * g:P * (g + 1)],
                                 lhsT=sel8[:, P * g:P * (g + 1)],
                                 rhs=qTs[:], start=True, stop=True)
            nc.vector.tensor_copy(out=rep[:, q, :], in_=rep_ps[:])
        x2R = rep[:, 0, :]; nx1R = rep[:, 1, :]
        y2R = rep[:, 2, :]; ny1R = rep[:, 3, :]; naR = rep[:, 4, :]

        # ---- I: L-matrix bits (all DVE, triangle-truncated) ----
        Lw_i = sb.tile([P, G, 64], I32)
        nc.vector.memset(Lw_i[:], 0)
        for g in range(G):
            K = P * (g + 1)
            sx2_s = srt40[:, 5 * g + 3:5 * g + 4]
            nsx1_s = qg[:, 1, g:g + 1]
            sy2_s = srt40[:, 5 * g + 4:5 * g + 5]
            nsy1_s = qg[:, 3, g:g + 1]
            a_s = ar[:, g:g + 1]
            a2 = big1.tile([P, C], F32, name="a2")
            nc.vector.tensor_scalar(out=a2[:, 0:K], in0=nx1R[:, 0:K], scalar1=nsx1_s,
                                    scalar2=None, op0=OP.min)
            iw = big2.tile([P, C], F32, name="iw")
            nc.vector.scalar_tensor_tensor(out=iw[:, 0:K], in0=x2R[:, 0:K], scalar=sx2_s,
                                           op0=OP.min, in1=a2[:, 0:K], op1=OP.add)
            b2 = big1.tile([P, C], F32, name="b2")
            nc.vector.tensor_scalar(out=b2[:, 0:K], in0=ny1R[:, 0:K], scalar1=nsy1_s,
                                    scalar2=None, op0=OP.min)
            ih = big1.tile([P, C], F32, name="ih")
            nc.vector.scalar_tensor_tensor(out=ih[:, 0:K], in0=y2R[:, 0:K], scalar=sy2_s,
                                           op0=OP.min, in1=b2[:, 0:K], op1=OP.add)
            nc.vector.tensor_scalar(out=ih[:, 0:K], in0=ih[:, 0:K], scalar1=0.0,
                                    scalar2=3.0, op0=OP.max, op1=OP.mult)
            nc.vector.tensor_tensor(out=iw[:, 0:K], in0=iw[:, 0:K], in1=ih[:, 0:K],
                                    op=OP.mult)
            nc.vector.tensor_tensor(out=iw[:, 0:K], in0=iw[:, 0:K], in1=naR[:, 0:K],
                                    op=OP.add)
            bt = big2.tile([P, C], F32, name="bt")
            nc.vector.scalar_tensor_tensor(
                out=bt[:, 0:K], in0=iw[:, 0:K], scalar=a_s, op0=OP.is_gt,
                in1=pwext[:, C - P * g:C - P * g + K], op1=OP.mult)
            with nc.allow_low_precision(reason="exact int sums <= 65535 in i32"):
                nc.vector.tensor_reduce(
                    out=Lw_i[:, g, 0:8 * (g + 1)],
                    in_=bt[:, 0:K].rearrange("p (w b) -> p w b", b=16), axis=AX.X,
                    op=OP.add)
        if stop_after == "stageI":
            lwf = sb.tile([P, 5], F32, name="lwf")
            nc.vector.tensor_copy(out=lwf[:], in_=Lw_i[:, 0, 0:5])
            nc.sync.dma_start(out=out_d[0:P, :], in_=lwf[:])
            return

        # ---- J: greedy fixed point ----
        kvec = svalid
        for r in range(R_GREEDY):
            rhs2 = sb.tile([P, 64], F32, name="rhs2")
            nc.vector.tensor_tensor(
                out=rhs2[:].rearrange("p (g m) -> p g m", g=G),
                in0=kvec[:].rearrange("p (g o) -> p g o", o=1).to_broadcast([P, G, 8]),
                in1=wconst[:].rearrange("p (g m) -> p g m", g=G), op=OP.mult)
            pk = psC.tile([1, 64], F32, space="PSUM", tag="pk")
            nc.tensor.matmul(out=pk[:], lhsT=onescol[:], rhs=rhs2[:],
                             start=True, stop=True)
            rowk = sb.tile([1, 64], F32, name="rowk")
            nc.vector.tensor_copy(out=rowk[:], in_=pk[:])
            bcp = psC.tile([P, 64], F32, space="PSUM", tag="bcp")
            nc.tensor.matmul(out=bcp[:], lhsT=onesrow[:], rhs=rowk[:],
                             start=True, stop=True)
            kw = sb.tile([P, 64], I32, name="kw")
            nc.vector.tensor_copy(out=kw[:], in_=bcp[:])
            tmp = sb.tile([P, G, 64], I32, name="gtmp")
            nc.vector.tensor_tensor(
                out=tmp[:], in0=Lw_i[:],
                in1=kw[:].rearrange("p (o w) -> p o w", o=1).to_broadcast([P, G, 64]),
                op=OP.bitwise_and)
            red = sb.tile([P, G], I32, name="gred")
            nc.vector.tensor_reduce(out=red[:], in_=tmp[:], axis=AX.X, op=OP.max)
            kvec = sb.tile([P, G], F32, name="kv")
            nc.vector.scalar_tensor_tensor(
                out=kvec[:], in0=red[:], scalar=0, op0=OP.is_equal,
                in1=svalid[:], op1=OP.mult)
        if stop_after == "greedy":
            nc.sync.dma_start(out=out_d[0:P, 0:5], in_=kvec[:, 0:5])
            return

        # ---- K: output ----
        outrow = sb.tile([P, G, 5], F32)
        nc.vector.tensor_tensor(out=outrow[:, :, 0], in0=ss, in1=kvec[:], op=OP.mult)
        nc.vector.tensor_tensor(out=outrow[:, :, 1], in0=sx1, in1=kvec[:], op=OP.mult)
        nc.vector.tensor_tensor(out=outrow[:, :, 2], in0=sy1, in1=kvec[:], op=OP.mult)
        nc.vector.tensor_tensor(out=outrow[:, :, 3], in0=arw[:], in1=kvec[:], op=OP.mult)
        nc.vector.tensor_tensor(out=outrow[:, :, 4], in0=arh[:], in1=kvec[:], op=OP.mult)
        outv = out_d[0:C, :].rearrange("(g p) q -> p g q", p=P)
        nc.sync.dma_start(out=outv[:, 0:4], in_=outrow[:, 0:4, :])
        nc.scalar.dma_start(out=outv[:, 4:8], in_=outrow[:, 4:8, :])


_CACHED = {}


def _get_nc():
    if "nc" not in _CACHED:
        _CACHED["nc"] = build()
        _CACHED["consts"] = host_constants()
    return _CACHED["nc"], _CACHED["consts"]


def kernel(outs0, outs1, np0=40, np1=80, **_ignored):
    import numpy as _np
    from concourse.bass_utils import run_bass_kernel_spmd

    outs0 = _np.ascontiguousarray(_np.asarray(outs0, dtype=_np.float32))
    outs1 = _np.ascontiguousarray(_np.asarray(outs1, dtype=_np.float32))
    assert outs0.shape == (5, 40, 40) and outs1.shape == (5, 80, 80)
    nc, consts = _get_nc()
    in_map = {"outs0": outs0, "outs1": outs1}
    in_map.update(consts)
    res = run_bass_kernel_spmd(nc, [dict(in_map) for _ in range(8)], list(range(8)))
    return _np.asarray(res.results[0]["out"], dtype=_np.float32)
